# revision 2
# baseline (speedup 1.0000x reference)
"""Trainium2 Bass kernel v2 for BatchRemoveQuatDiscontinuities.

Algorithm (per (batch, joint) lane):
    d[t]    = dot(q[t], q[t-1])            (fp32, 4-wide dot)
    flip[t] = 1 if d[t] < 0 else 0         (t >= 1; flip[0] = 0)
    sigma[t] = (-1)^(sum_{s<=t} flip[s])   (cumulative sign parity)
    out[t]  = q[t] * sigma[t]

v2 changes vs v1 (155us baseline):
  * Output is bf16 (rel-err gate is 2e-2; bf16 adds ~2e-3): halves the
    output DMA traffic (24MB/core total -> ~70us HBM floor).
  * Input DMA on the SP HWDGE ring, output DMA on the ACT ring, so
    output waits can't stall input prefetch (FIFO per ring).
  * Work split DVE/GpSimd by tunable ranges; the d<0 test fuses the
    final pair-add via scalar_tensor_tensor: e = (u1*-1) is_gt u0,
    bit-exact to fl(u0+u1)<0 (near-cancellation adds are exact).
  * The octet-offset matmul accumulates onto a PSUM tile pre-seeded
    (ACT copy) with the within-octet scan parity, so no sigma-combine
    tensor_tensor is needed; parity n -> sigma in {+1,-1} via one ACT
    op (Sin(pi*n + pi/2) = cos(pi*n)), with mod/int fallbacks.
"""

import numpy as np
from contextlib import ExitStack

import concourse.bass as bass
import concourse.bacc as bacc
import concourse.tile as tile
from concourse import mybir
from concourse.bass_utils import run_bass_kernel_spmd

B, T, J, C = 128, 1024, 64, 4
NCORES = 8
JC = J * C                      # 256 floats per t
BPC = B // NCORES               # 16 batch clips per core
TS = 8                          # t per partition (octet)
FD = TS * JC                    # tile free dim = 2048 floats
SD = J * TS                     # prefix free dim = 512 (j, ts)

FP32 = mybir.dt.float32
BF16 = mybir.dt.bfloat16
I32 = mybir.dt.int32
Alu = mybir.AluOpType
Act = mybir.ActivationFunctionType
PI = 3.14159265358979


def _ap(apx, dims, extra_offset=0):
    """AP with explicit [step, count] free dims appended to partition dim."""
    return bass.AP(
        tensor=apx.tensor, offset=apx.offset + extra_offset,
        ap=[list(apx.ap[0]), *[list(d) for d in dims]],
    )


def build_nc(bpc=BPC, t=T, reps=1, mode="full",
             na=7, nb=0, nj=6, ce="v", parity="sin", out_dt="bf16",
             debug=False):
    """na: A-main 256-chunks on DVE (0..7); nb: B 128-chunks on DVE (0..8);
    nj: J ts-chunks on DVE (0..8); ce: 'v'|'g' engine for the fused
    add+compare; parity: 'sin'|'mod'|'int'."""
    assert t % (128 * TS) == 0
    ODT = BF16 if out_dt == "bf16" else FP32
    nc = bacc.Bacc(None, target_bir_lowering=False)
    q = nc.declare_dram_parameter("q", [bpc, t, J, C], FP32, isOutput=False)
    smat = nc.declare_dram_parameter("smat", [128, 128], FP32, isOutput=False)
    pmat = nc.declare_dram_parameter("pmat", [128, 128], FP32, isOutput=False)
    out = nc.declare_dram_parameter("out", [bpc, t, J, C], ODT, isOutput=True)
    qf = q.rearrange("b t j c -> b (t j c)")
    of = out.rearrange("b t j c -> b (t j c)")
    if debug:
        dbg_u = nc.declare_dram_parameter("dbg_u", [bpc, 128, 2 * SD], FP32,
                                          isOutput=True)
        dbg_e = nc.declare_dram_parameter("dbg_e", [bpc, 128, SD], BF16,
                                          isOutput=True)
        dbg_rowp = nc.declare_dram_parameter("dbg_rowp", [bpc, 128, SD], FP32,
                                             isOutput=True)
        dbg_pt = nc.declare_dram_parameter("dbg_pt", [bpc, 128, J], FP32,
                                           isOutput=True)
        dbg_sig = nc.declare_dram_parameter("dbg_sig", [bpc, 128, SD], BF16,
                                            isOutput=True)

    with tile.TileContext(nc) as tc, ExitStack() as ctx:
        consts = ctx.enter_context(tc.tile_pool(name="consts", bufs=1))
        qpool = ctx.enter_context(tc.tile_pool(name="qpool", bufs=6))
        opool = ctx.enter_context(tc.tile_pool(name="opool", bufs=3))
        upool = ctx.enter_context(tc.tile_pool(name="upool", bufs=3))
        spool = ctx.enter_context(tc.tile_pool(name="spool", bufs=4))
        obpool = ctx.enter_context(tc.tile_pool(name="obpool", bufs=4))
        auxp = ctx.enter_context(tc.tile_pool(name="auxp", bufs=2, space="PSUM"))
        ptp = ctx.enter_context(tc.tile_pool(name="ptp", bufs=2, space="PSUM"))

        smatSB = consts.tile([128, 128], FP32)
        nc.sync.dma_start(out=smatSB[:, :], in_=smat[:, :])
        pmatSB = consts.tile([128, 128], FP32)
        nc.sync.dma_start(out=pmatSB[:, :], in_=pmat[:, :])
        amask = consts.tile([128, SD], FP32)
        nc.vector.memset(amask[:, :], 1.0)
        nc.vector.memset(
            amask.rearrange("p (j ts) -> p j ts", ts=TS)[:, :, 0], 0.0
        )

        def emit_tile(b):
            qt = qpool.tile([128, FD], FP32, tag="qt")
            nc.sync.dma_start(
                out=qt[:, :],
                in_=qf[b, :].rearrange("(p x) -> p x", p=128),
            )
            ob = obpool.tile([128, FD], ODT, tag="ob")
            if mode == "dma":
                nc.scalar.dma_start(
                    out=of[b, :].rearrange("(p x) -> p x", p=128), in_=qt[:, :]
                )
                return

            # octet-boundary shift: aux[p] = qt[p-1, ts=7 chunk] (row 0 = 0)
            aux = auxp.tile([128, JC], FP32, tag="aux")
            nc.tensor.matmul(
                aux[:, :], lhsT=smatSB[:, :], rhs=qt[:, FD - JC:FD],
                start=True, stop=True,
            )

            # A: products o = q * q_shifted (boundary chunk from PSUM on DVE)
            o = opool.tile([128, FD], FP32, tag="o")
            nc.vector.tensor_tensor(
                out=o[:, 0:JC], in0=qt[:, 0:JC], in1=aux[:, :], op=Alu.mult,
            )
            asp = JC + na * JC  # DVE takes [JC, asp), GpSimd [asp, FD)
            if na > 0:
                nc.vector.tensor_tensor(
                    out=o[:, JC:asp], in0=qt[:, JC:asp],
                    in1=qt[:, 0:asp - JC], op=Alu.mult,
                )
            if na < 7:
                nc.gpsimd.tensor_tensor(
                    out=o[:, asp:FD], in0=qt[:, asp:FD],
                    in1=qt[:, asp - JC:FD - JC], op=Alu.mult,
                )

            # B: pairwise add u[s,k] = o[s,2k] + o[s,2k+1]  (1024 wide)
            u = upool.tile([128, 2 * SD], FP32, tag="u")
            uv = u.rearrange("p (s k) -> p s k", k=2)
            ov = o.rearrange("p (s c) -> p s c", c=C)
            opairs = ov.rearrange("p s (k two) -> p s k two", k=2)
            bsp = nb * 128  # of 1024, in u-elements (s-major)
            if nb > 0:
                nc.vector.tensor_tensor(
                    out=uv[:, 0:bsp // 2, :],
                    in0=opairs[:, 0:bsp // 2, :, 0],
                    in1=opairs[:, 0:bsp // 2, :, 1], op=Alu.add,
                )
            if nb < 8:
                nc.gpsimd.tensor_tensor(
                    out=uv[:, bsp // 2:SD, :],
                    in0=opairs[:, bsp // 2:SD, :, 0],
                    in1=opairs[:, bsp // 2:SD, :, 1], op=Alu.add,
                )

            # CE: e = ((u1 * -1) > u0) == (fl(u0+u1) < 0); write (j, ts)
            # layout for the segmented scan.  u is (ts, j, k=2) order.
            e = spool.tile([128, SD], BF16, tag="e")
            e_t = _ap(e, [[1, TS], [TS, J]])          # iterate (ts, j)
            u0 = _ap(u, [[2 * J, TS], [2, J]])        # u[.., k=0]
            u1 = _ap(u, [[2 * J, TS], [2, J]], 1)     # u[.., k=1]
            eng = nc.vector if ce == "v" else nc.gpsimd
            eng.scalar_tensor_tensor(
                out=e_t, in0=u1, scalar=-1.0, in1=u0,
                op0=Alu.mult, op1=Alu.is_gt,
            )

            # within-octet inclusive prefix PARITY (segmented xor-scan)
            rowp = spool.tile([128, SD], FP32, tag="rowp")
            nc.vector.tensor_tensor_scan(
                out=rowp[:, :], data0=amask[:, :], data1=e[:, :],
                initial=0.0, op0=Alu.mult, op1=Alu.logical_xor,
            )

            # octet-level: count of odd octets above (parity-sum via matmul)
            offs = ptp.tile([128, J], FP32, tag="offs")
            nc.tensor.matmul(
                offs[:, :], lhsT=pmatSB[:, :],
                rhs=rowp.rearrange("p (j ts) -> p j ts", ts=TS)[:, :, TS - 1],
                start=True, stop=True,
            )
            # parity of that count -> sigma_off in {+1, -1} per (p, j)
            offi = spool.tile([128, J], I32, tag="offi")
            nc.vector.tensor_copy(out=offi[:, :], in_=offs[:, :])
            offb = spool.tile([128, J], I32, tag="offb")
            nc.vector.tensor_scalar(
                out=offb[:, :], in0=offi[:, :], scalar1=1, scalar2=None,
                op0=Alu.bitwise_and,
            )
            sigo = spool.tile([128, J], BF16, tag="sigo")
            nc.scalar.activation(sigo[:, :], offb[:, :], Act.Copy,
                                 bias=1.0, scale=-2.0)
            # sigma_row in {+1, -1} from the 0/1 row parity
            sigr = spool.tile([128, SD], BF16, tag="sigr")
            nc.scalar.activation(sigr[:, :], rowp[:, :], Act.Copy,
                                 bias=1.0, scale=-2.0)
            # sigma = sigma_row * sigma_off, (j, ts) layout
            sig = spool.tile([128, SD], BF16, tag="sig")
            cseng = nc.vector if parity != "g" else nc.gpsimd
            cseng.tensor_tensor(
                out=sig[:, :], in0=sigr[:, :],
                in1=_ap(sigo, [[1, J], [0, TS]]),
                op=Alu.mult,
            )

            # out = q * sigma (broadcast over c), split DVE/GpSimd by ts
            qv = qt.rearrange("p (ts x) -> p ts x", ts=TS)
            ow = ob.rearrange("p (ts x) -> p ts x", ts=TS)
            if nj > 0:
                nc.vector.tensor_tensor(
                    out=ow[:, 0:nj, :], in0=qv[:, 0:nj, :],
                    in1=_ap(sig, [[1, nj], [TS, J], [0, C]]),
                    op=Alu.mult,
                )
            if nj < TS:
                nc.gpsimd.tensor_tensor(
                    out=ow[:, nj:TS, :], in0=qv[:, nj:TS, :],
                    in1=_ap(sig, [[1, TS - nj], [TS, J], [0, C]], nj),
                    op=Alu.mult,
                )

            nc.scalar.dma_start(
                out=of[b, :].rearrange("(p x) -> p x", p=128), in_=ob[:, :]
            )
            if debug:
                nc.sync.dma_start(out=dbg_u[b], in_=u[:, :])
                nc.sync.dma_start(out=dbg_e[b], in_=e[:, :])
                nc.sync.dma_start(out=dbg_rowp[b], in_=rowp[:, :])
                ptc = spool.tile([128, J], FP32, tag="ptc")
                nc.vector.tensor_copy(out=ptc[:, :], in_=offs[:, :])
                nc.sync.dma_start(out=dbg_pt[b], in_=ptc[:, :])
                nc.sync.dma_start(out=dbg_sig[b], in_=sig[:, :])

        def emit_body():
            for b in range(bpc):
                emit_tile(b)

        if reps == 1:
            emit_body()
        else:
            with tc.For_i(0, reps, 1):
                emit_body()
    return nc


def make_consts():
    smat = np.eye(128, k=1, dtype=np.float32)       # S[k, m] = 1 iff m == k+1
    pmat = np.triu(np.ones((128, 128), np.float32), k=1)  # strict prefix
    return smat, pmat


def kernel(joint_rotations: np.ndarray) -> np.ndarray:
    q = np.ascontiguousarray(joint_rotations, dtype=np.float32)
    assert q.shape == (B, T, J, C)
    smat, pmat = make_consts()
    nc = build_nc()
    nc.finalize()   # run bacc passes (wait splitting, reg alloc) + freeze
    in_maps = [
        {"q": q[c * BPC:(c + 1) * BPC], "smat": smat, "pmat": pmat}
        for c in range(NCORES)
    ]
    res = run_bass_kernel_spmd(nc, in_maps, list(range(NCORES)))
    outs = [np.asarray(r["out"]).astype(np.float32) for r in res.results]
    return np.concatenate(outs, axis=0)


# revision 3
# speedup vs baseline: 1.0556x; 1.0556x over previous
"""Trainium2 Bass kernel v2 for BatchRemoveQuatDiscontinuities.

Algorithm (per (batch, joint) lane):
    d[t]    = dot(q[t], q[t-1])            (fp32, 4-wide dot)
    flip[t] = 1 if d[t] < 0 else 0         (t >= 1; flip[0] = 0)
    sigma[t] = (-1)^(sum_{s<=t} flip[s])   (cumulative sign parity)
    out[t]  = q[t] * sigma[t]

v2 changes vs v1 (155us baseline):
  * Output is bf16 (rel-err gate is 2e-2; bf16 adds ~2e-3): halves the
    output DMA traffic (24MB/core total -> ~70us HBM floor).
  * Input DMA on the SP HWDGE ring, output DMA on the ACT ring, so
    output waits can't stall input prefetch (FIFO per ring).
  * Work split DVE/GpSimd by tunable ranges; the d<0 test fuses the
    final pair-add via scalar_tensor_tensor: e = (u1*-1) is_gt u0,
    bit-exact to fl(u0+u1)<0 (near-cancellation adds are exact).
  * Octet-level sign: strict-triangular matmul counts odd octets above,
    int cast + &1 gives its parity, ACT affines produce sigma_off /
    sigma_row in {+1,-1}, one broadcast tensor_tensor combines them.
"""

import numpy as np
from contextlib import ExitStack

import concourse.bass as bass
import concourse.bacc as bacc
import concourse.tile as tile
from concourse import mybir
from concourse.bass_utils import run_bass_kernel_spmd

B, T, J, C = 128, 1024, 64, 4
NCORES = 8
JC = J * C                      # 256 floats per t
BPC = B // NCORES               # 16 batch clips per core
TS = 8                          # t per partition (octet)
FD = TS * JC                    # tile free dim = 2048 floats
SD = J * TS                     # prefix free dim = 512 (j, ts)

FP32 = mybir.dt.float32
BF16 = mybir.dt.bfloat16
I32 = mybir.dt.int32
Alu = mybir.AluOpType
Act = mybir.ActivationFunctionType
PI = 3.14159265358979


def _ap(apx, dims, extra_offset=0):
    """AP with explicit [step, count] free dims appended to partition dim."""
    return bass.AP(
        tensor=apx.tensor, offset=apx.offset + extra_offset,
        ap=[list(apx.ap[0]), *[list(d) for d in dims]],
    )


def build_nc(bpc=BPC, t=T, reps=1, mode="full",
             na=7, nb=0, nj=6, ce="v", parity="sin", out_dt="bf16",
             debug=False):
    """na: A-main 256-chunks on DVE (0..7); nb: B 128-chunks on DVE (0..8);
    nj: J ts-chunks on DVE (0..8); ce: 'v'|'g' engine for the fused
    add+compare; parity: 'sin'|'mod'|'int'."""
    assert t % (128 * TS) == 0
    ODT = BF16 if out_dt == "bf16" else FP32
    nc = bacc.Bacc(None, target_bir_lowering=False)
    q = nc.declare_dram_parameter("q", [bpc, t, J, C], FP32, isOutput=False)
    smat = nc.declare_dram_parameter("smat", [128, 128], FP32, isOutput=False)
    pmat = nc.declare_dram_parameter("pmat", [128, 128], FP32, isOutput=False)
    out = nc.declare_dram_parameter("out", [bpc, t, J, C], ODT, isOutput=True)
    qf = q.rearrange("b t j c -> b (t j c)")
    of = out.rearrange("b t j c -> b (t j c)")
    if debug:
        dbg_u = nc.declare_dram_parameter("dbg_u", [bpc, 128, 2 * SD], FP32,
                                          isOutput=True)
        dbg_e = nc.declare_dram_parameter("dbg_e", [bpc, 128, SD], BF16,
                                          isOutput=True)
        dbg_rowp = nc.declare_dram_parameter("dbg_rowp", [bpc, 128, SD], FP32,
                                             isOutput=True)
        dbg_pt = nc.declare_dram_parameter("dbg_pt", [bpc, 128, J], FP32,
                                           isOutput=True)
        dbg_sig = nc.declare_dram_parameter("dbg_sig", [bpc, 128, SD], BF16,
                                            isOutput=True)

    with tile.TileContext(nc) as tc, ExitStack() as ctx:
        consts = ctx.enter_context(tc.tile_pool(name="consts", bufs=1))
        qpool = ctx.enter_context(tc.tile_pool(name="qpool", bufs=6))
        opool = ctx.enter_context(tc.tile_pool(name="opool", bufs=3))
        upool = ctx.enter_context(tc.tile_pool(name="upool", bufs=3))
        spool = ctx.enter_context(tc.tile_pool(name="spool", bufs=4))
        obpool = ctx.enter_context(tc.tile_pool(name="obpool", bufs=4))
        auxp = ctx.enter_context(tc.tile_pool(name="auxp", bufs=2, space="PSUM"))
        ptp = ctx.enter_context(tc.tile_pool(name="ptp", bufs=2, space="PSUM"))

        smatSB = consts.tile([128, 128], FP32)
        nc.sync.dma_start(out=smatSB[:, :], in_=smat[:, :])
        pmatSB = consts.tile([128, 128], FP32)
        nc.sync.dma_start(out=pmatSB[:, :], in_=pmat[:, :])
        amask = consts.tile([128, SD], FP32)
        nc.vector.memset(amask[:, :], 1.0)
        nc.vector.memset(
            amask.rearrange("p (j ts) -> p j ts", ts=TS)[:, :, 0], 0.0
        )

        def emit_tile(b):
            qt = qpool.tile([128, FD], FP32, tag="qt")
            nc.sync.dma_start(
                out=qt[:, :],
                in_=qf[b, :].rearrange("(p x) -> p x", p=128),
            )
            ob = obpool.tile([128, FD], ODT, tag="ob")
            if mode == "dma":
                nc.scalar.dma_start(
                    out=of[b, :].rearrange("(p x) -> p x", p=128), in_=qt[:, :]
                )
                return

            # octet-boundary shift: aux[p] = qt[p-1, ts=7 chunk] (row 0 = 0)
            aux = auxp.tile([128, JC], FP32, tag="aux")
            nc.tensor.matmul(
                aux[:, :], lhsT=smatSB[:, :], rhs=qt[:, FD - JC:FD],
                start=True, stop=True,
            )

            # A: products o = q * q_shifted (boundary chunk from PSUM on DVE)
            o = opool.tile([128, FD], FP32, tag="o")
            nc.vector.tensor_tensor(
                out=o[:, 0:JC], in0=qt[:, 0:JC], in1=aux[:, :], op=Alu.mult,
            )
            asp = JC + na * JC  # DVE takes [JC, asp), GpSimd [asp, FD)
            if na > 0:
                nc.vector.tensor_tensor(
                    out=o[:, JC:asp], in0=qt[:, JC:asp],
                    in1=qt[:, 0:asp - JC], op=Alu.mult,
                )
            if na < 7:
                nc.gpsimd.tensor_tensor(
                    out=o[:, asp:FD], in0=qt[:, asp:FD],
                    in1=qt[:, asp - JC:FD - JC], op=Alu.mult,
                )

            # B: pairwise add u[s,k] = o[s,2k] + o[s,2k+1]  (1024 wide)
            u = upool.tile([128, 2 * SD], FP32, tag="u")
            uv = u.rearrange("p (s k) -> p s k", k=2)
            ov = o.rearrange("p (s c) -> p s c", c=C)
            opairs = ov.rearrange("p s (k two) -> p s k two", k=2)
            bsp = nb * 128  # of 1024, in u-elements (s-major)
            if nb > 0:
                nc.vector.tensor_tensor(
                    out=uv[:, 0:bsp // 2, :],
                    in0=opairs[:, 0:bsp // 2, :, 0],
                    in1=opairs[:, 0:bsp // 2, :, 1], op=Alu.add,
                )
            if nb < 8:
                nc.gpsimd.tensor_tensor(
                    out=uv[:, bsp // 2:SD, :],
                    in0=opairs[:, bsp // 2:SD, :, 0],
                    in1=opairs[:, bsp // 2:SD, :, 1], op=Alu.add,
                )

            # CE: e = ((u1 * -1) > u0) == (fl(u0+u1) < 0); write (j, ts)
            # layout for the segmented scan.  u is (ts, j, k=2) order.
            e = spool.tile([128, SD], BF16, tag="e")
            e_t = _ap(e, [[1, TS], [TS, J]])          # iterate (ts, j)
            u0 = _ap(u, [[2 * J, TS], [2, J]])        # u[.., k=0]
            u1 = _ap(u, [[2 * J, TS], [2, J]], 1)     # u[.., k=1]
            eng = nc.vector if ce == "v" else nc.gpsimd
            eng.scalar_tensor_tensor(
                out=e_t, in0=u1, scalar=-1.0, in1=u0,
                op0=Alu.mult, op1=Alu.is_gt,
            )

            # within-octet inclusive prefix PARITY (segmented xor-scan)
            rowp = spool.tile([128, SD], FP32, tag="rowp")
            nc.vector.tensor_tensor_scan(
                out=rowp[:, :], data0=amask[:, :], data1=e[:, :],
                initial=0.0, op0=Alu.mult, op1=Alu.logical_xor,
            )

            # octet-level: count of odd octets above (parity-sum via matmul)
            offs = ptp.tile([128, J], FP32, tag="offs")
            nc.tensor.matmul(
                offs[:, :], lhsT=pmatSB[:, :],
                rhs=rowp.rearrange("p (j ts) -> p j ts", ts=TS)[:, :, TS - 1],
                start=True, stop=True,
            )
            # parity of that count -> sigma_off in {+1, -1} per (p, j)
            offi = spool.tile([128, J], I32, tag="offi")
            nc.vector.tensor_copy(out=offi[:, :], in_=offs[:, :])
            offb = spool.tile([128, J], I32, tag="offb")
            nc.vector.tensor_scalar(
                out=offb[:, :], in0=offi[:, :], scalar1=1, scalar2=None,
                op0=Alu.bitwise_and,
            )
            sigo = spool.tile([128, J], BF16, tag="sigo")
            nc.scalar.activation(sigo[:, :], offb[:, :], Act.Copy,
                                 bias=1.0, scale=-2.0)
            # sigma_row in {+1, -1} from the 0/1 row parity
            sigr = spool.tile([128, SD], BF16, tag="sigr")
            nc.scalar.activation(sigr[:, :], rowp[:, :], Act.Copy,
                                 bias=1.0, scale=-2.0)
            # sigma = sigma_row * sigma_off, (j, ts) layout
            sig = spool.tile([128, SD], BF16, tag="sig")
            cseng = nc.vector if parity != "g" else nc.gpsimd
            cseng.tensor_tensor(
                out=sig[:, :], in0=sigr[:, :],
                in1=_ap(sigo, [[1, J], [0, TS]]),
                op=Alu.mult,
            )

            # out = q * sigma (broadcast over c), split DVE/GpSimd by ts
            qv = qt.rearrange("p (ts x) -> p ts x", ts=TS)
            ow = ob.rearrange("p (ts x) -> p ts x", ts=TS)
            if nj > 0:
                nc.vector.tensor_tensor(
                    out=ow[:, 0:nj, :], in0=qv[:, 0:nj, :],
                    in1=_ap(sig, [[1, nj], [TS, J], [0, C]]),
                    op=Alu.mult,
                )
            if nj < TS:
                nc.gpsimd.tensor_tensor(
                    out=ow[:, nj:TS, :], in0=qv[:, nj:TS, :],
                    in1=_ap(sig, [[1, TS - nj], [TS, J], [0, C]], nj),
                    op=Alu.mult,
                )

            nc.scalar.dma_start(
                out=of[b, :].rearrange("(p x) -> p x", p=128), in_=ob[:, :]
            )
            if debug:
                nc.sync.dma_start(out=dbg_u[b], in_=u[:, :])
                nc.sync.dma_start(out=dbg_e[b], in_=e[:, :])
                nc.sync.dma_start(out=dbg_rowp[b], in_=rowp[:, :])
                ptc = spool.tile([128, J], FP32, tag="ptc")
                nc.vector.tensor_copy(out=ptc[:, :], in_=offs[:, :])
                nc.sync.dma_start(out=dbg_pt[b], in_=ptc[:, :])
                nc.sync.dma_start(out=dbg_sig[b], in_=sig[:, :])

        def emit_body():
            for b in range(bpc):
                emit_tile(b)

        if reps == 1:
            emit_body()
        else:
            with tc.For_i(0, reps, 1):
                emit_body()
    return nc


def make_consts():
    smat = np.eye(128, k=1, dtype=np.float32)       # S[k, m] = 1 iff m == k+1
    pmat = np.triu(np.ones((128, 128), np.float32), k=1)  # strict prefix
    return smat, pmat


def kernel(joint_rotations: np.ndarray) -> np.ndarray:
    q = np.ascontiguousarray(joint_rotations, dtype=np.float32)
    assert q.shape == (B, T, J, C)
    smat, pmat = make_consts()
    nc = build_nc()
    nc.finalize()   # run bacc passes (wait splitting, reg alloc) + freeze
    in_maps = [
        {"q": q[c * BPC:(c + 1) * BPC], "smat": smat, "pmat": pmat}
        for c in range(NCORES)
    ]
    res = run_bass_kernel_spmd(nc, in_maps, list(range(NCORES)))
    outs = [np.asarray(r["out"]).astype(np.float32) for r in res.results]
    return np.concatenate(outs, axis=0)
